# revision 10
# baseline (speedup 1.0000x reference)
"""LoRA linear (y = x @ (W + s*B@A)^T + bias) on 8 Trainium2 NeuronCores.

Strategy: pure data parallel over the token dim; LoRA folded into the weight
on the host (W' = W + 4 * B @ A); x / W' / bias cast to bf16 on the host
(fro rel-err ~3e-3, far inside the 2e-2 gate) which halves all DMA traffic
and keeps the PE at 1 col/cycle (same rate as fp32r, half the bytes).

Per core: out[2048, 1024](bf16) = xT[:, shard].T @ wT + bias
  - w [128(d), 8*1024] and x [128(d), 8*2048] bf16 resident in SBUF; bias
    pre-replicated on host to [128, 1024] bf16. All DMA on the single sync
    HWDGE ring in exact consumption order (issue cost ~600 ns each, ring
    drains FIFO).
  - ~7 warm-up matmuls on scratch bf16 tiles bridge the gap between the
    engine preamble (~6.9 us) and first operand arrival (~9.7 us) so the
    HAM clock gate is at 8/8 before the real stream starts (else the first
    ~3.4 us of matmuls run at 1.2 GHz).
  - PSUM as 8 single-bank tiles [128(n), 512(o)].
    Group 0 (tokens 0:512) runs d-outer so each arriving (w[d], x[d]) pair
    immediately enables 8 matmuls.
    Tokens 512:2048 run as a job pipeline: one job = (128 tokens, 512 outs,
    all 8 d) = 8 matmuls into one bank, then evict. Evictions are spaced
    every ~1.7 us instead of bursting 8 at each group boundary, so a bank
    is always free ~13 us before its reuse -> no PE stalls.
  - Evictions (DVE/Pool tensor_add: psum fp32 + bias bf16 -> bf16 out)
    alternate between vector and gpsimd so eviction throughput (~2x 0.5 us
    per half) outpaces matmul bank turnover; out DMA rides the same sync
    ring (input stream is finished by the time output volume matters).
"""

import os
import sys

import numpy as np

for _p in ("/opt/trn_rl_repo", "/opt/pypackages"):
    if os.path.isdir(_p) and _p not in sys.path:
        sys.path.append(_p)

try:
    import jax

    jax.config.update(
        "jax_compilation_cache_dir", os.path.expanduser("~/.cache/jax_bass_cache")
    )
    jax.config.update("jax_persistent_cache_min_compile_time_secs", 0.0)
except Exception:
    pass

try:
    # bass_utils imports this when tracing is requested via BASS_TRACE; the
    # agent image ships a stub antenv without it. Register a no-op fallback
    # so a trace request degrades to "no trace" instead of crashing.
    from antenv import axon_hooks as _axon_hooks  # noqa: F401
except ImportError:
    import types as _types

    import antenv as _antenv

    _hooks = _types.ModuleType("antenv.axon_hooks")
    _hooks._hook = None
    _hooks.set_axon_ntff_profile_hook = lambda h: setattr(_hooks, "_hook", h)
    _hooks.get_axon_ntff_profile_hook = lambda: _hooks._hook
    sys.modules["antenv.axon_hooks"] = _hooks
    _antenv.axon_hooks = _hooks

import ml_dtypes  # noqa: E402

import concourse.bass as bass  # noqa: E402,F401
import concourse.mybir as mybir  # noqa: E402
import concourse.tile as tile  # noqa: E402
from concourse import bacc  # noqa: E402
from concourse.bass_utils import run_bass_kernel_spmd  # noqa: E402

N_CORES = 8
N_TOK, D_IN, D_OUT = 16384, 1024, 1024
N_SHARD = N_TOK // N_CORES  # 2048 tokens per core
P = 128
SCALING = 4.0  # alpha / r = 32 / 8

_CACHE: dict = {}


def build_nc():
    f32 = mybir.dt.float32
    bf16 = mybir.dt.bfloat16
    nc = bacc.Bacc("TRN2", target_bir_lowering=False, debug=False)

    xT = nc.dram_tensor("xT", [D_IN, N_SHARD], bf16, kind="ExternalInput")
    wT = nc.dram_tensor("wT", [D_IN, D_OUT], bf16, kind="ExternalInput")
    bias = nc.dram_tensor("bias", [P, D_OUT], bf16, kind="ExternalInput")
    out = nc.dram_tensor("out", [N_SHARD, D_OUT], bf16, kind="ExternalOutput")

    KT = D_IN // P  # 8 contraction tiles
    NBLK = 512  # tokens per group
    OH = 512  # one PSUM bank of fp32 output per matmul
    NH = D_OUT // OH  # 2 output halves

    with tile.TileContext(nc) as tc:
        with tc.tile_pool(name="const", bufs=1) as const_pool, \
                tc.tile_pool(name="ps", bufs=8, space="PSUM") as psum_pool:
            x_sb = const_pool.tile([P, KT * N_SHARD], bf16, name="x_sb")
            w_sb = const_pool.tile([P, KT * D_OUT], bf16, name="w_sb")
            bias_sb = const_pool.tile([P, D_OUT], bf16, name="bias_sb")

            def xsl(kt, t0, t1):
                return x_sb[:, kt * N_SHARD + t0:kt * N_SHARD + t1]

            def wsl(kt, o0, o1):
                return w_sb[:, kt * D_OUT + o0:kt * D_OUT + o1]

            # Warm-up scratch (zeroed so the PE never streams NaN garbage).
            # N=128 warm-ups (~107 ns cold each) bridge preamble-end to
            # operand arrival at fine granularity: real work is delayed at
            # most one warm-up when data lands.
            warm_x = const_pool.tile([P, P], bf16, name="warm_x")
            warm_w = const_pool.tile([P, OH], bf16, name="warm_w")
            nc.gpsimd.memset(warm_x[:], 0.0)
            nc.gpsimd.memset(warm_w[:], 0.0)
            warm_ps = psum_pool.tile([P, OH], f32, name="warm_ps", tag="psum")
            for _ in range(18):
                nc.tensor.matmul(warm_ps[:, 0:P], warm_x[:], warm_w[:, 0:P],
                                 start=True, stop=True)

            # Startup stream in exact consumption order (single sync ring).
            # x slice first so the first LDWEIGHTS fires earliest.
            nc.sync.dma_start(xsl(0, 0, P), xT[0:P, 0:P])
            nc.scalar.dma_start(wsl(0, 0, OH), wT[0:P, 0:OH])
            nc.sync.dma_start(xsl(0, P, NBLK), xT[0:P, P:NBLK])
            nc.scalar.dma_start(wsl(0, OH, D_OUT), wT[0:P, OH:D_OUT])
            for t in range(1, KT):
                nc.sync.dma_start(xsl(t, 0, NBLK), xT[t * P:(t + 1) * P, 0:NBLK])
                nc.scalar.dma_start(
                    w_sb[:, t * D_OUT:(t + 1) * D_OUT], wT[t * P:(t + 1) * P, :]
                )
            nc.scalar.dma_start(bias_sb[:], bias[:])
            # Remaining tokens, needed from ~22 us onward.
            for t in range(KT):
                nc.sync.dma_start(
                    xsl(t, NBLK, N_SHARD), xT[t * P:(t + 1) * P, NBLK:N_SHARD]
                )

            def evict(n0, h, psum, o_sb, quarters=False):
                # DVE eviction (GPSIMD cannot read PSUM). The job pipeline
                # retires one [128,512] eviction (~0.7 us) per 8-matmul job
                # (~1.7 us), so DVE alone keeps banks free well before
                # reuse. psum is a single [128, OH] bank; the out/bias
                # slices carry the h offset.
                pieces = 2 if quarters else 1
                step = OH // pieces
                for q in range(pieces):
                    psl = slice(q * step, (q + 1) * step)
                    qsl = slice(h * OH + q * step, h * OH + (q + 1) * step)
                    nc.vector.tensor_add(o_sb[:, qsl], psum[:, psl],
                                         bias_sb[:, qsl])
                    nc.scalar.dma_start(out[n0:n0 + P, qsl], o_sb[:, qsl])

            # ---- Group 0 (tokens 0:512): d-outer over 8 single-bank psums.
            g0_ps = [
                psum_pool.tile([P, OH], f32, name=f"ps_g0_{i}_{h}", tag="psum")
                for i in range(4) for h in range(NH)
            ]
            g0_osb = [const_pool.tile([P, D_OUT], bf16, name=f"o_g0_{i}")
                      for i in range(4)]
            for d in range(KT):
                for i in range(4):
                    lhsT = xsl(d, i * P, (i + 1) * P)
                    for h in range(NH):
                        nc.tensor.matmul(
                            g0_ps[i * NH + h][:],
                            lhsT,
                            wsl(d, h * OH, (h + 1) * OH),
                            start=(d == 0),
                            stop=(d == KT - 1),
                        )
            for i in range(4):
                for h in range(NH):
                    evict(i * P, h, g0_ps[i * NH + h], g0_osb[i])

            # ---- Tokens 512:2048: job pipeline. One job = (128 tokens,
            # 512 outs, all 8 d) into one psum bank, then evict.
            jobs = [
                (NBLK + j // NH * P, j % NH)  # (token offset, out half)
                for j in range(((N_SHARD - NBLK) // P) * NH)
            ]
            n_jobs = len(jobs)
            osb_map = {}
            for j, (n0, h) in enumerate(jobs):
                if h == 0:
                    osb_map[n0] = const_pool.tile([P, D_OUT], bf16,
                                                  name=f"o_j{n0}")
                ps = psum_pool.tile([P, OH], f32, name=f"ps_j{j}", tag="psum")
                if j < n_jobs - 1:
                    for d in range(KT):
                        nc.tensor.matmul(
                            ps[:],
                            xsl(d, n0, n0 + P),
                            wsl(d, h * OH, (h + 1) * OH),
                            start=(d == 0),
                            stop=(d == KT - 1),
                        )
                    evict(n0, h, ps, osb_map[n0])
                else:
                    # Last job: two independent N=256 chains so the first
                    # half evicts ~0.9 us before the final matmul and the
                    # exposed tail is a single [128,256] add + DMA.
                    Q = OH // 2
                    o_sb = osb_map[n0]
                    for cq in range(2):
                        o0 = h * OH + cq * Q
                        for d in range(KT):
                            nc.tensor.matmul(
                                ps[:, cq * Q:(cq + 1) * Q],
                                xsl(d, n0, n0 + P),
                                wsl(d, o0, o0 + Q),
                                start=(d == 0),
                                stop=(d == KT - 1),
                            )
                        nc.vector.tensor_add(
                            o_sb[:, o0:o0 + Q],
                            ps[:, cq * Q:(cq + 1) * Q],
                            bias_sb[:, o0:o0 + Q],
                        )
                        nc.scalar.dma_start(out[n0:n0 + P, o0:o0 + Q],
                                            o_sb[:, o0:o0 + Q])

    nc.finalize()
    return nc


def _get_nc():
    if "nc" not in _CACHE:
        _CACHE["nc"] = build_nc()
    return _CACHE["nc"]


def kernel(x, weight, bias, A, B):
    x = np.asarray(x, dtype=np.float32)
    weight = np.asarray(weight, dtype=np.float32)
    bias = np.asarray(bias, dtype=np.float32)
    A = np.asarray(A, dtype=np.float32)
    B = np.asarray(B, dtype=np.float32)

    # Fold the rank-8 LoRA update into the weight (exact up to rounding).
    w_eff = (
        weight.astype(np.float64)
        + SCALING * (B.astype(np.float64) @ A.astype(np.float64))
    ).astype(np.float32)
    wT = np.ascontiguousarray(w_eff.T.astype(ml_dtypes.bfloat16))  # [d, o]
    xT = np.ascontiguousarray(x.T.astype(ml_dtypes.bfloat16))  # [d, n]
    bias_rep = np.ascontiguousarray(
        np.broadcast_to(bias.astype(ml_dtypes.bfloat16), (P, D_OUT))
    )

    nc = _get_nc()
    in_maps = [
        {
            "xT": np.ascontiguousarray(xT[:, c * N_SHARD:(c + 1) * N_SHARD]),
            "wT": wT,
            "bias": bias_rep,
        }
        for c in range(N_CORES)
    ]
    trace_kwargs = {}
    if os.environ.get("KERNEL_TRACE") == "1":
        trace_kwargs = {"trace": True}
    res = run_bass_kernel_spmd(nc, in_maps, list(range(N_CORES)), **trace_kwargs)
    _CACHE["last_results"] = res
    return np.concatenate(
        [r["out"].astype(np.float32) for r in res.results], axis=0
    )


# revision 11
# speedup vs baseline: 1.0132x; 1.0132x over previous
"""LoRA linear (y = x @ (W + s*B@A)^T + bias) on 8 Trainium2 NeuronCores.

Strategy: pure data parallel over the token dim; LoRA folded into the weight
on the host (W' = W + 4 * B @ A); x / W' / bias cast to bf16 on the host
(fro rel-err ~3e-3, far inside the 2e-2 gate) which halves all DMA traffic
and keeps the PE at 1 col/cycle (same rate as fp32r, half the bytes).

Per core: out[2048, 1024](bf16) = xT[:, shard].T @ wT + bias
  - w [128(d), 8*1024] and x [128(d), 8*2048] bf16 resident in SBUF; bias
    pre-replicated on host to [128, 1024] bf16. All DMA on the single sync
    HWDGE ring in exact consumption order (issue cost ~600 ns each, ring
    drains FIFO).
  - ~7 warm-up matmuls on scratch bf16 tiles bridge the gap between the
    engine preamble (~6.9 us) and first operand arrival (~9.7 us) so the
    HAM clock gate is at 8/8 before the real stream starts (else the first
    ~3.4 us of matmuls run at 1.2 GHz).
  - PSUM as 8 single-bank tiles [128(n), 512(o)].
    Group 0 (tokens 0:512) runs d-outer so each arriving (w[d], x[d]) pair
    immediately enables 8 matmuls.
    Tokens 512:2048 run as a job pipeline: one job = (128 tokens, 512 outs,
    all 8 d) = 8 matmuls into one bank, then evict. Evictions are spaced
    every ~1.7 us instead of bursting 8 at each group boundary, so a bank
    is always free ~13 us before its reuse -> no PE stalls.
  - Evictions (DVE/Pool tensor_add: psum fp32 + bias bf16 -> bf16 out)
    alternate between vector and gpsimd so eviction throughput (~2x 0.5 us
    per half) outpaces matmul bank turnover; out DMA rides the same sync
    ring (input stream is finished by the time output volume matters).
"""

import os
import sys

import numpy as np

for _p in ("/opt/trn_rl_repo", "/opt/pypackages"):
    if os.path.isdir(_p) and _p not in sys.path:
        sys.path.append(_p)

try:
    import jax

    jax.config.update(
        "jax_compilation_cache_dir", os.path.expanduser("~/.cache/jax_bass_cache")
    )
    jax.config.update("jax_persistent_cache_min_compile_time_secs", 0.0)
except Exception:
    pass

try:
    # bass_utils imports this when tracing is requested via BASS_TRACE; the
    # agent image ships a stub antenv without it. Register a no-op fallback
    # so a trace request degrades to "no trace" instead of crashing.
    from antenv import axon_hooks as _axon_hooks  # noqa: F401
except ImportError:
    import types as _types

    import antenv as _antenv

    _hooks = _types.ModuleType("antenv.axon_hooks")
    _hooks._hook = None
    _hooks.set_axon_ntff_profile_hook = lambda h: setattr(_hooks, "_hook", h)
    _hooks.get_axon_ntff_profile_hook = lambda: _hooks._hook
    sys.modules["antenv.axon_hooks"] = _hooks
    _antenv.axon_hooks = _hooks

import ml_dtypes  # noqa: E402

import concourse.bass as bass  # noqa: E402,F401
import concourse.mybir as mybir  # noqa: E402
import concourse.tile as tile  # noqa: E402
from concourse import bacc  # noqa: E402
from concourse.bass_utils import run_bass_kernel_spmd  # noqa: E402

N_CORES = 8
N_TOK, D_IN, D_OUT = 16384, 1024, 1024
N_SHARD = N_TOK // N_CORES  # 2048 tokens per core
P = 128
SCALING = 4.0  # alpha / r = 32 / 8

_CACHE: dict = {}


def build_nc():
    f32 = mybir.dt.float32
    bf16 = mybir.dt.bfloat16
    nc = bacc.Bacc("TRN2", target_bir_lowering=False, debug=False)

    xT = nc.dram_tensor("xT", [D_IN, N_SHARD], bf16, kind="ExternalInput")
    wT = nc.dram_tensor("wT", [D_IN, D_OUT], bf16, kind="ExternalInput")
    bias = nc.dram_tensor("bias", [P, D_OUT], bf16, kind="ExternalInput")
    out = nc.dram_tensor("out", [N_SHARD, D_OUT], bf16, kind="ExternalOutput")

    KT = D_IN // P  # 8 contraction tiles
    NBLK = 512  # tokens per group
    OH = 512  # one PSUM bank of fp32 output per matmul
    NH = D_OUT // OH  # 2 output halves

    with tile.TileContext(nc) as tc:
        with tc.tile_pool(name="const", bufs=1) as const_pool, \
                tc.tile_pool(name="ps", bufs=8, space="PSUM") as psum_pool:
            x_sb = const_pool.tile([P, KT * N_SHARD], bf16, name="x_sb")
            w_sb = const_pool.tile([P, KT * D_OUT], bf16, name="w_sb")
            bias_sb = const_pool.tile([P, D_OUT], bf16, name="bias_sb")

            def xsl(kt, t0, t1):
                return x_sb[:, kt * N_SHARD + t0:kt * N_SHARD + t1]

            def wsl(kt, o0, o1):
                return w_sb[:, kt * D_OUT + o0:kt * D_OUT + o1]

            # Warm-up scratch (zeroed so the PE never streams NaN garbage).
            # N=128 warm-ups (~107 ns cold each) bridge preamble-end to
            # operand arrival at fine granularity: real work is delayed at
            # most one warm-up when data lands.
            warm_x = const_pool.tile([P, P], bf16, name="warm_x")
            warm_w = const_pool.tile([P, OH], bf16, name="warm_w")
            nc.gpsimd.memset(warm_x[:], 0.0)
            nc.gpsimd.memset(warm_w[:], 0.0)
            warm_ps = psum_pool.tile([P, OH], f32, name="warm_ps", tag="psum")
            for _ in range(16):
                nc.tensor.matmul(warm_ps[:, 0:P], warm_x[:], warm_w[:, 0:P],
                                 start=True, stop=True)

            # Startup stream in exact consumption order (single sync ring).
            # x slice first so the first LDWEIGHTS fires earliest.
            # First pair split across both HWDGE rings so the x and w
            # issue latencies overlap; everything else stays on sync in
            # consumption order (the scalar ring competes packet-by-packet
            # with sync for the shared SDMA engines, so keeping bulk w
            # loads there starves the stream).
            nc.sync.dma_start(xsl(0, 0, P), xT[0:P, 0:P])
            nc.scalar.dma_start(wsl(0, 0, OH), wT[0:P, 0:OH])
            nc.sync.dma_start(xsl(0, P, NBLK), xT[0:P, P:NBLK])
            nc.scalar.dma_start(wsl(0, OH, D_OUT), wT[0:P, OH:D_OUT])
            for t in range(1, KT):
                nc.sync.dma_start(xsl(t, 0, NBLK), xT[t * P:(t + 1) * P, 0:NBLK])
                nc.sync.dma_start(
                    w_sb[:, t * D_OUT:(t + 1) * D_OUT], wT[t * P:(t + 1) * P, :]
                )
            nc.scalar.dma_start(bias_sb[:], bias[:])
            # Remaining tokens, needed from ~22 us onward.
            for t in range(KT):
                nc.sync.dma_start(
                    xsl(t, NBLK, N_SHARD), xT[t * P:(t + 1) * P, NBLK:N_SHARD]
                )

            def evict(n0, h, psum, o_sb, quarters=False):
                # DVE eviction (GPSIMD cannot read PSUM). The job pipeline
                # retires one [128,512] eviction (~0.7 us) per 8-matmul job
                # (~1.7 us), so DVE alone keeps banks free well before
                # reuse. psum is a single [128, OH] bank; the out/bias
                # slices carry the h offset.
                pieces = 2 if quarters else 1
                step = OH // pieces
                for q in range(pieces):
                    psl = slice(q * step, (q + 1) * step)
                    qsl = slice(h * OH + q * step, h * OH + (q + 1) * step)
                    nc.vector.tensor_add(o_sb[:, qsl], psum[:, psl],
                                         bias_sb[:, qsl])
                    nc.scalar.dma_start(out[n0:n0 + P, qsl], o_sb[:, qsl])

            # ---- Group 0 (tokens 0:512): d-outer over 8 single-bank psums.
            g0_ps = [
                psum_pool.tile([P, OH], f32, name=f"ps_g0_{i}_{h}", tag="psum")
                for i in range(4) for h in range(NH)
            ]
            g0_osb = [const_pool.tile([P, D_OUT], bf16, name=f"o_g0_{i}")
                      for i in range(4)]
            for d in range(KT):
                for i in range(4):
                    lhsT = xsl(d, i * P, (i + 1) * P)
                    for h in range(NH):
                        nc.tensor.matmul(
                            g0_ps[i * NH + h][:],
                            lhsT,
                            wsl(d, h * OH, (h + 1) * OH),
                            start=(d == 0),
                            stop=(d == KT - 1),
                        )
            for i in range(4):
                for h in range(NH):
                    evict(i * P, h, g0_ps[i * NH + h], g0_osb[i])

            # ---- Tokens 512:2048: job pipeline. One job = (128 tokens,
            # 512 outs, all 8 d) into one psum bank, then evict.
            jobs = [
                (NBLK + j // NH * P, j % NH)  # (token offset, out half)
                for j in range(((N_SHARD - NBLK) // P) * NH)
            ]
            n_jobs = len(jobs)
            osb_map = {}
            for j, (n0, h) in enumerate(jobs):
                if h == 0:
                    osb_map[n0] = const_pool.tile([P, D_OUT], bf16,
                                                  name=f"o_j{n0}")
                ps = psum_pool.tile([P, OH], f32, name=f"ps_j{j}", tag="psum")
                if j < n_jobs - 1:
                    for d in range(KT):
                        nc.tensor.matmul(
                            ps[:],
                            xsl(d, n0, n0 + P),
                            wsl(d, h * OH, (h + 1) * OH),
                            start=(d == 0),
                            stop=(d == KT - 1),
                        )
                    evict(n0, h, ps, osb_map[n0])
                else:
                    # Last job: two independent N=256 chains so the first
                    # half evicts ~0.9 us before the final matmul and the
                    # exposed tail is a single [128,256] add + DMA.
                    Q = OH // 2
                    o_sb = osb_map[n0]
                    for cq in range(2):
                        o0 = h * OH + cq * Q
                        for d in range(KT):
                            nc.tensor.matmul(
                                ps[:, cq * Q:(cq + 1) * Q],
                                xsl(d, n0, n0 + P),
                                wsl(d, o0, o0 + Q),
                                start=(d == 0),
                                stop=(d == KT - 1),
                            )
                        nc.vector.tensor_add(
                            o_sb[:, o0:o0 + Q],
                            ps[:, cq * Q:(cq + 1) * Q],
                            bias_sb[:, o0:o0 + Q],
                        )
                        nc.scalar.dma_start(out[n0:n0 + P, o0:o0 + Q],
                                            o_sb[:, o0:o0 + Q])

    nc.finalize()
    return nc


def _get_nc():
    if "nc" not in _CACHE:
        _CACHE["nc"] = build_nc()
    return _CACHE["nc"]


def kernel(x, weight, bias, A, B):
    x = np.asarray(x, dtype=np.float32)
    weight = np.asarray(weight, dtype=np.float32)
    bias = np.asarray(bias, dtype=np.float32)
    A = np.asarray(A, dtype=np.float32)
    B = np.asarray(B, dtype=np.float32)

    # Fold the rank-8 LoRA update into the weight (exact up to rounding).
    w_eff = (
        weight.astype(np.float64)
        + SCALING * (B.astype(np.float64) @ A.astype(np.float64))
    ).astype(np.float32)
    wT = np.ascontiguousarray(w_eff.T.astype(ml_dtypes.bfloat16))  # [d, o]
    xT = np.ascontiguousarray(x.T.astype(ml_dtypes.bfloat16))  # [d, n]
    bias_rep = np.ascontiguousarray(
        np.broadcast_to(bias.astype(ml_dtypes.bfloat16), (P, D_OUT))
    )

    nc = _get_nc()
    in_maps = [
        {
            "xT": np.ascontiguousarray(xT[:, c * N_SHARD:(c + 1) * N_SHARD]),
            "wT": wT,
            "bias": bias_rep,
        }
        for c in range(N_CORES)
    ]
    trace_kwargs = {}
    if os.environ.get("KERNEL_TRACE") == "1":
        trace_kwargs = {"trace": True}
    res = run_bass_kernel_spmd(nc, in_maps, list(range(N_CORES)), **trace_kwargs)
    _CACHE["last_results"] = res
    return np.concatenate(
        [r["out"].astype(np.float32) for r in res.results], axis=0
    )
